# revision 12
# baseline (speedup 1.0000x reference)
"""CorrelationOnlyTracker on 8 TRN2 NeuronCores (Bass/Tile, SPMD).

Pipeline (two SPMD launches, sharded by the 48 = B*T frames, 6 per core):
  Launch A (encoder): host pre-builds a stride-baked im2col of the video for
    conv1 (K-stacked taps -> dense 126/21-partition matmuls); conv2/conv3 run
    from zero-padded SBUF layouts via strided access patterns. Features
    [C=256, 32x32] per frame are written back to HBM.
  Host glue: bilinear-samples the query feature vectors (tiny gather) and
    transposes them for the PE.
  Launch B (correlation): each core computes, for its 6 frames and the 512
    queries of the owning batch, the q . x correlation via matmul, then
    softmax / soft-argmax / occlusion on ACT+DVE, entirely on-chip.

Matmuls run as float32r (full-rate fp32 mode, N=512 moving tiles).
"""

import os
import numpy as np
import ml_dtypes

BF = ml_dtypes.bfloat16
from contextlib import ExitStack

import concourse.bass as bass
import concourse.tile as tile
from concourse import bacc, mybir
from concourse.bass_utils import run_bass_kernel_spmd

AF = mybir.ActivationFunctionType
AX = mybir.AxisListType
F32 = mybir.dt.float32
F32R = mybir.dt.float32r
BF16 = mybir.dt.bfloat16

N_CORES = 8
FPC = 6          # frames per core (48 frames total)
B, T, H, W = 2, 24, 256, 256
NQ = 512
FH = FW = 32
C = 256

# last-run instrumentation (filled when BASS_TRACE is set)
LAST_RUN_INFO = {}


def _mm(ap):
    return ap


TAPS9 = [(ky, kx) for ky in range(3) for kx in range(3)]


def build_encoder():
    nc = bacc.Bacc("TRN2", target_bir_lowering=False, debug=False, num_devices=N_CORES)
    xcolA = nc.dram_tensor("xcolA", [FPC, 126, 128, 132], BF16, kind="ExternalInput")
    xcolB = nc.dram_tensor("xcolB", [FPC, 21, 128, 132], BF16, kind="ExternalInput")
    w1A = nc.dram_tensor("w1A", [126, 64], BF16, kind="ExternalInput")
    w1B = nc.dram_tensor("w1B", [21, 64], BF16, kind="ExternalInput")
    w2s = nc.dram_tensor("w2s", [64, 9, 128], BF16, kind="ExternalInput")
    w3s = nc.dram_tensor("w3s", [128, 9, 256], BF16, kind="ExternalInput")
    biasv = nc.dram_tensor("biasv", [128, 4], F32, kind="ExternalInput")
    feat = nc.dram_tensor("feat", [FPC, 128, 2, 1024], BF16, kind="ExternalOutput")

    with tile.TileContext(nc) as tc, ExitStack() as ctx:
        consts = ctx.enter_context(tc.tile_pool(name="consts", bufs=1))
        xa_pool = ctx.enter_context(tc.tile_pool(name="xa", bufs=2))
        xb_pool = ctx.enter_context(tc.tile_pool(name="xb", bufs=2))
        act_pool = ctx.enter_context(tc.tile_pool(name="acts", bufs=1))
        feat_pool = ctx.enter_context(tc.tile_pool(name="featsb", bufs=2))
        ps_pool = ctx.enter_context(tc.tile_pool(name="ps", bufs=6, space="PSUM"))

        w1a_sb = consts.tile([126, 64], BF16)
        w1b_sb = consts.tile([21, 64], BF16)
        w2_sb = consts.tile([64, 9, 128], BF16)
        w3_sb = consts.tile([128, 9, 256], BF16)
        bias_sb = consts.tile([128, 4], F32)
        nc.sync.dma_start(w1a_sb[:], w1A[:])
        nc.sync.dma_start(w1b_sb[:], w1B[:])
        nc.sync.dma_start(w2_sb[:], w2s[:])
        nc.sync.dma_start(w3_sb[:], w3s[:])
        nc.sync.dma_start(bias_sb[:], biasv[:])

        # persistent padded activation planes; pad regions must stay zero
        x1pad = act_pool.tile([64, 129, 132], BF16)   # conv1 out 128x128 (pad 0,1)
        x2pad = act_pool.tile([128, 65, 68], BF16)    # conv2 out 64x64 (pad 0,1)
        nc.vector.memset(x1pad[:], 0.0)
        nc.vector.memset(x2pad[:], 0.0)

        for f in range(FPC):
            # ---- conv1: 7x7 s2, K stacked across (ky,kx,c) taps ----
            for s in range(8):              # strips of 16 output rows
                xa = xa_pool.tile([126, 16, 132], BF16)
                xb = xb_pool.tile([21, 16, 132], BF16)
                nc.sync.dma_start(xa[:], xcolA[f, :, 16 * s:16 * s + 16, :])
                nc.sync.dma_start(xb[:], xcolB[f, :, 16 * s:16 * s + 16, :])
                for nt in range(4):         # 4 output rows x 128 cols = 512
                    r0 = 4 * nt
                    ps = ps_pool.tile([64, 512], F32, tag="ps")
                    nc.tensor.matmul(ps[:], _mm(w1a_sb[:]),
                                     _mm(xa[:, r0:r0 + 4, 0:128]),
                                     start=True, stop=False)
                    nc.tensor.matmul(ps[:], _mm(w1b_sb[:]),
                                     _mm(xb[:, r0:r0 + 4, 0:128]),
                                     start=False, stop=True)
                    gr = 16 * s + r0
                    nc.scalar.activation(x1pad[:, gr:gr + 4, 0:128], ps[:],
                                         AF.Relu, bias=bias_sb[0:64, 0:1])
            # ---- conv2: 3x3 s2, K=64, taps via strided APs on x1pad ----
            for nt in range(8):             # 8 output rows x 64 cols = 512
                ps = ps_pool.tile([128, 512], F32, tag="ps")
                for tap, (ky, kx) in enumerate(TAPS9):
                    base = 16 * nt + ky
                    rhs = x1pad[:, base:base + 15:2, kx:kx + 127:2]  # [64,8,64]
                    nc.tensor.matmul(ps[:], _mm(w2_sb[:, tap, :]), _mm(rhs),
                                     start=(tap == 0), stop=(tap == 8))
                nc.scalar.activation(x2pad[:, 8 * nt:8 * nt + 8, 0:64], ps[:],
                                     AF.Relu, bias=bias_sb[:, 1:2])
            # ---- conv3: 3x3 s2, K=128 ----
            fs = feat_pool.tile([128, 2, 1024], BF16)
            for mt in range(2):
                for nt2 in range(2):        # 16 output rows x 32 cols = 512
                    ps = ps_pool.tile([128, 512], F32, tag="ps")
                    for tap, (ky, kx) in enumerate(TAPS9):
                        base = 32 * nt2 + ky
                        rhs = x2pad[:, base:base + 31:2, kx:kx + 63:2]  # [128,16,32]
                        nc.tensor.matmul(ps[:],
                                         _mm(w3_sb[:, tap, 128 * mt:128 * mt + 128]),
                                         _mm(rhs),
                                         start=(tap == 0), stop=(tap == 8))
                    nc.scalar.activation(fs[:, mt, 512 * nt2:512 * nt2 + 512], ps[:],
                                         AF.Relu, bias=bias_sb[:, 2 + mt:3 + mt])
            nc.sync.dma_start(feat[f], fs[:])
    nc.compile()
    return nc


def build_correlator():
    nc = bacc.Bacc("TRN2", target_bir_lowering=False, debug=False, num_devices=N_CORES)
    feats = nc.dram_tensor("feats", [FPC, 128, 2, 1024], BF16, kind="ExternalInput")
    qTd = nc.dram_tensor("qTd", [128, 2, 512], BF16, kind="ExternalInput")
    cxy = nc.dram_tensor("cxy", [128, 64], F32, kind="ExternalInput")
    pred_out = nc.dram_tensor("pred_out", [4, 128, FPC, 2], F32, kind="ExternalOutput")
    occ_out = nc.dram_tensor("occ_out", [4, 128, FPC], F32, kind="ExternalOutput")

    with tile.TileContext(nc) as tc, ExitStack() as ctx:
        consts = ctx.enter_context(tc.tile_pool(name="consts", bufs=1))
        fpool = ctx.enter_context(tc.tile_pool(name="fpool", bufs=2))
        ppool = ctx.enter_context(tc.tile_pool(name="probs", bufs=2))
        spool = ctx.enter_context(tc.tile_pool(name="small", bufs=3))
        opool = ctx.enter_context(tc.tile_pool(name="outs", bufs=1))
        ps_pool = ctx.enter_context(tc.tile_pool(name="ps", bufs=3, space="PSUM"))

        qt_sb = consts.tile([128, 2, 512], BF16)
        cxy_sb = consts.tile([128, 64], F32)
        nc.sync.dma_start(qt_sb[:], qTd[:])
        nc.sync.dma_start(cxy_sb[:], cxy[:])
        predt = [opool.tile([128, FPC, 2], F32, tag=f"pred{i}", name=f"pred{i}")
                 for i in range(4)]
        occt = [opool.tile([128, FPC], F32, tag=f"occ{i}", name=f"occ{i}")
                for i in range(4)]

        for t in range(FPC):
            fsb = fpool.tile([128, 2, 1024], BF16)
            nc.sync.dma_start(fsb[:], feats[t])
            for qt in range(4):
                ps = ps_pool.tile([128, 1024], F32, tag="ps")
                for nh in range(2):
                    for kt in range(2):
                        nc.tensor.matmul(
                            ps[:, 512 * nh:512 * nh + 512],
                            _mm(qt_sb[:, kt, 128 * qt:128 * qt + 128]),
                            _mm(fsb[:, kt, 512 * nh:512 * nh + 512]),
                            start=(kt == 0), stop=(kt == 1))
                mx = spool.tile([128, 1], F32, tag="mx")
                nc.vector.reduce_max(mx[:], ps[:], axis=AX.X)
                # occlusion = sigmoid(-max(corr)/16)
                nc.scalar.activation(occt[qt][:, t:t + 1], mx[:], AF.Sigmoid,
                                     scale=-0.0625)
                nb = spool.tile([128, 1], F32, tag="nb")
                nc.scalar.mul(nb[:], mx[:], -0.625)
                probs = ppool.tile([128, 1024], F32, tag="probs")
                den = spool.tile([128, 1], F32, tag="den")
                # exp(10/16 * (corr - max)), row sums fused
                nc.scalar.activation(probs[:], ps[:], AF.Exp, bias=nb[:],
                                     scale=0.625, accum_out=den[:])
                margs = spool.tile([128, 64], F32, tag="margs")
                nc.vector.reduce_sum(margs[:, 32:64],
                                     probs[:].rearrange("p (h w) -> p h w", h=32),
                                     axis=AX.X)
                nc.vector.reduce_sum(margs[:, 0:32],
                                     probs[:].rearrange("p (h w) -> p w h", h=32),
                                     axis=AX.X)
                wm = spool.tile([128, 64], F32, tag="wm")
                nc.vector.tensor_mul(wm[:], margs[:], cxy_sb[:])
                num2 = spool.tile([128, 2], F32, tag="num2")
                nc.vector.reduce_sum(num2[:],
                                     wm[:].rearrange("p (a c) -> p a c", a=2),
                                     axis=AX.X)
                rec = spool.tile([128, 1], F32, tag="rec")
                nc.vector.reciprocal(rec[:], den[:])
                pr2 = spool.tile([128, 2], F32, tag="pr2")
                nc.vector.tensor_mul(pr2[:], num2[:],
                                     rec[:].to_broadcast([128, 2]))
                nc.scalar.mul(predt[qt][:, t, :], pr2[:], 8.0)
        for qt in range(4):
            nc.sync.dma_start(pred_out[qt], predt[qt][:])
            nc.sync.dma_start(occ_out[qt], occt[qt][:])
    nc.compile()
    return nc


_NC_CACHE = {}


def _get_nc(name):
    if name not in _NC_CACHE:
        _NC_CACHE[name] = build_encoder() if name == "enc" else build_correlator()
    return _NC_CACHE[name]


def _run(nc, in_maps, label):
    import time
    t0 = time.monotonic()
    res = run_bass_kernel_spmd(nc, in_maps, list(range(N_CORES)))
    LAST_RUN_INFO[label] = {"wall_ns": int((time.monotonic() - t0) * 1e9)}
    return res.results


def kernel(video, query_points, w1, b1, w2, b2, w3, b3):
    video = np.ascontiguousarray(np.asarray(video, np.float32))
    qp = np.asarray(query_points, np.float32)

    # ---- host: stride-baked im2col for conv1 (p = ky*21 + kx*3 + c) ----
    vpad = np.zeros((48, 262, 262, 3), np.float32)
    vpad[:, 2:258, 2:258, :] = video.reshape(48, 256, 256, 3)
    xcol = np.zeros((48, 147, 128, 132), BF)
    for ky in range(7):
        for kx in range(7):
            sl = vpad[:, ky:ky + 256:2, kx:kx + 256:2, :]      # [48,128,128,3]
            p0 = (ky * 7 + kx) * 3
            xcol[:, p0:p0 + 3, :, :128] = sl.transpose(0, 3, 1, 2).astype(BF)

    w1f = np.asarray(w1, np.float32)[0] / np.float32(255.0)    # fold /255
    w1r = w1f.reshape(147, 64)
    w2v = np.ascontiguousarray(
        np.asarray(w2, np.float32)[0].transpose(2, 0, 1, 3).reshape(64, 9, 128))
    w3v = np.ascontiguousarray(
        np.asarray(w3, np.float32)[0].transpose(2, 0, 1, 3).reshape(128, 9, 256))
    biasv = np.zeros((128, 4), np.float32)
    biasv[:64, 0] = b1
    biasv[:, 1] = b2
    biasv[:, 2] = np.asarray(b3)[:128]
    biasv[:, 3] = np.asarray(b3)[128:]

    in_maps_A = []
    for c in range(N_CORES):
        f0 = FPC * c
        in_maps_A.append({
            "xcolA": np.ascontiguousarray(xcol[f0:f0 + FPC, :126]),
            "xcolB": np.ascontiguousarray(xcol[f0:f0 + FPC, 126:]),
            "w1A": np.ascontiguousarray(w1r[:126]).astype(BF),
            "w1B": np.ascontiguousarray(w1r[126:]).astype(BF),
            "w2s": w2v.astype(BF), "w3s": w3v.astype(BF), "biasv": biasv,
        })
    resA = _run(_get_nc("enc"), in_maps_A, "encoder")
    feat_all = np.concatenate([resA[i]["feat"] for i in range(N_CORES)], axis=0)
    # feat_all: [48, c_lo(128), c_tile(2), hw(1024)]

    # ---- host: bilinear sample query feature vectors ----
    x = feat_all.transpose(0, 3, 2, 1).reshape(48, 32, 32, 256).astype(np.float32)
    x = x.reshape(B, T, FH, FW, C)
    tq = np.clip((qp[:, :, 0] * (T - 1)).astype(np.int32), 0, T - 1)
    yq = qp[:, :, 1] * np.float32(FH - 1)
    xq = qp[:, :, 2] * np.float32(FW - 1)
    y0 = np.clip(np.floor(yq).astype(np.int32), 0, FH - 1)
    y1 = np.clip(y0 + 1, 0, FH - 1)
    x0 = np.clip(np.floor(xq).astype(np.int32), 0, FW - 1)
    x1 = np.clip(x0 + 1, 0, FW - 1)
    wy1 = (yq - y0).astype(np.float32)[..., None]
    wx1 = (xq - x0).astype(np.float32)[..., None]
    wy0 = 1.0 - wy1
    wx0 = 1.0 - wx1
    bi = np.arange(B)[:, None]
    f00 = x[bi, tq, y0, x0]
    f01 = x[bi, tq, y0, x1]
    f10 = x[bi, tq, y1, x0]
    f11 = x[bi, tq, y1, x1]
    q = (f00 * wx0 + f01 * wx1) * wy0 + (f10 * wx0 + f11 * wx1) * wy1  # [B,NQ,C]

    qTd = np.stack([
        np.ascontiguousarray(q[b].T.reshape(2, 128, 512).transpose(1, 0, 2))
        for b in range(B)
    ]).astype(BF)  # [B, 128, 2, 512]
    cxyv = np.tile(np.concatenate([np.arange(32, dtype=np.float32)] * 2)[None], (128, 1))

    in_maps_B = []
    for c in range(N_CORES):
        f0 = FPC * c
        in_maps_B.append({
            "feats": np.ascontiguousarray(feat_all[f0:f0 + FPC]),
            "qTd": qTd[f0 // T],
            "cxy": cxyv,
        })
    resB = _run(_get_nc("corr"), in_maps_B, "correlator")

    pred = np.zeros((B, NQ, T, 2), np.float32)
    occ = np.zeros((B, NQ, T), np.float32)
    for c in range(N_CORES):
        b = (FPC * c) // T
        t0 = (FPC * c) % T
        pred[b, :, t0:t0 + FPC, :] = resB[c]["pred_out"].reshape(NQ, FPC, 2)
        occ[b, :, t0:t0 + FPC] = resB[c]["occ_out"].reshape(NQ, FPC)
    return pred, occ


# revision 13
# speedup vs baseline: 1.5974x; 1.5974x over previous
"""CorrelationOnlyTracker on 8 TRN2 NeuronCores (Bass/Tile, SPMD).

Pipeline (two SPMD launches, sharded by the 48 = B*T frames, 6 per core):
  Launch A (encoder): host pre-builds a stride-baked im2col of the video for
    conv1 (K-stacked taps -> dense 126/21-partition matmuls); conv2/conv3 run
    from zero-padded SBUF layouts via strided access patterns. Features
    [C=256, 32x32] per frame are written back to HBM.
  Host glue: bilinear-samples the query feature vectors (tiny gather) and
    transposes them for the PE.
  Launch B (correlation): each core computes, for its 6 frames and the 512
    queries of the owning batch, the q . x correlation via matmul, then
    softmax / soft-argmax / occlusion on ACT+DVE, entirely on-chip.

Matmuls run as float32r (full-rate fp32 mode, N=512 moving tiles).
"""

import os
import numpy as np
import ml_dtypes

BF = ml_dtypes.bfloat16
from contextlib import ExitStack

import concourse.bass as bass
import concourse.tile as tile
from concourse import bacc, mybir
from concourse.bass_utils import run_bass_kernel_spmd

AF = mybir.ActivationFunctionType
AX = mybir.AxisListType
F32 = mybir.dt.float32
F32R = mybir.dt.float32r
BF16 = mybir.dt.bfloat16

N_CORES = 8
FPC = 6          # frames per core (48 frames total)
B, T, H, W = 2, 24, 256, 256
NQ = 512
FH = FW = 32
C = 256

# last-run instrumentation (filled when BASS_TRACE is set)
LAST_RUN_INFO = {}


def _mm(ap):
    return ap


TAPS9 = [(ky, kx) for ky in range(3) for kx in range(3)]


def build_encoder():
    nc = bacc.Bacc("TRN2", target_bir_lowering=False, debug=False, num_devices=N_CORES)
    xcolA = nc.dram_tensor("xcolA", [FPC, 126, 128, 132], BF16, kind="ExternalInput")
    xcolB = nc.dram_tensor("xcolB", [FPC, 21, 128, 132], BF16, kind="ExternalInput")
    w1A = nc.dram_tensor("w1A", [126, 64], BF16, kind="ExternalInput")
    w1B = nc.dram_tensor("w1B", [21, 64], BF16, kind="ExternalInput")
    w2s = nc.dram_tensor("w2s", [64, 9, 128], BF16, kind="ExternalInput")
    w3s = nc.dram_tensor("w3s", [128, 9, 256], BF16, kind="ExternalInput")
    biasv = nc.dram_tensor("biasv", [128, 4], F32, kind="ExternalInput")
    feat = nc.dram_tensor("feat", [FPC, 128, 2, 1024], BF16, kind="ExternalOutput")

    with tile.TileContext(nc) as tc, ExitStack() as ctx:
        consts = ctx.enter_context(tc.tile_pool(name="consts", bufs=1))
        xa_pool = ctx.enter_context(tc.tile_pool(name="xa", bufs=3))
        xb_pool = ctx.enter_context(tc.tile_pool(name="xb", bufs=3))
        act_pool = ctx.enter_context(tc.tile_pool(name="acts", bufs=1))
        feat_pool = ctx.enter_context(tc.tile_pool(name="featsb", bufs=2))
        ps_pool = ctx.enter_context(tc.tile_pool(name="ps", bufs=6, space="PSUM"))

        w1a_sb = consts.tile([126, 64], BF16)
        w1b_sb = consts.tile([21, 64], BF16)
        w2_sb = consts.tile([64, 9, 128], BF16)
        w3_sb = consts.tile([128, 9, 256], BF16)
        bias_sb = consts.tile([128, 4], F32)
        nc.sync.dma_start(w1a_sb[:], w1A[:])
        nc.sync.dma_start(w1b_sb[:], w1B[:])
        nc.sync.dma_start(w2_sb[:], w2s[:])
        nc.sync.dma_start(w3_sb[:], w3s[:])
        nc.sync.dma_start(bias_sb[:], biasv[:])

        # persistent padded activation planes; pad regions must stay zero
        x1pad = act_pool.tile([64, 129, 132], BF16)   # conv1 out 128x128 (pad 0,1)
        x2pad = act_pool.tile([128, 65, 68], BF16)    # conv2 out 64x64 (pad 0,1)
        nc.vector.memset(x1pad[:], 0.0)
        nc.vector.memset(x2pad[:], 0.0)

        for f in range(FPC):
            # ---- conv1: 7x7 s2, K stacked across (ky,kx,c) taps ----
            for s in range(8):              # strips of 16 output rows
                xa = xa_pool.tile([126, 16, 132], BF16)
                xb = xb_pool.tile([21, 16, 132], BF16)
                nc.sync.dma_start(xa[:], xcolA[f, :, 16 * s:16 * s + 16, :])
                nc.sync.dma_start(xb[:], xcolB[f, :, 16 * s:16 * s + 16, :])
                for nt in range(4):         # 4 output rows x 128 cols = 512
                    r0 = 4 * nt
                    ps = ps_pool.tile([64, 512], F32, tag="ps")
                    nc.tensor.matmul(ps[:], _mm(w1a_sb[:]),
                                     _mm(xa[:, r0:r0 + 4, 0:128]),
                                     start=True, stop=False)
                    nc.tensor.matmul(ps[:], _mm(w1b_sb[:]),
                                     _mm(xb[:, r0:r0 + 4, 0:128]),
                                     start=False, stop=True)
                    gr = 16 * s + r0
                    nc.scalar.activation(x1pad[:, gr:gr + 4, 0:128], ps[:],
                                         AF.Relu, bias=bias_sb[0:64, 0:1])
            # ---- conv2: 3x3 s2, K=64, taps via strided APs on x1pad ----
            for nt in range(8):             # 8 output rows x 64 cols = 512
                ps = ps_pool.tile([128, 512], F32, tag="ps")
                for tap, (ky, kx) in enumerate(TAPS9):
                    base = 16 * nt + ky
                    rhs = x1pad[:, base:base + 15:2, kx:kx + 127:2]  # [64,8,64]
                    nc.tensor.matmul(ps[:], _mm(w2_sb[:, tap, :]), _mm(rhs),
                                     start=(tap == 0), stop=(tap == 8))
                nc.scalar.activation(x2pad[:, 8 * nt:8 * nt + 8, 0:64], ps[:],
                                     AF.Relu, bias=bias_sb[:, 1:2])
            # ---- conv3: 3x3 s2, K=128 ----
            fs = feat_pool.tile([128, 2, 1024], BF16)
            for mt in range(2):
                for nt2 in range(2):        # 16 output rows x 32 cols = 512
                    ps = ps_pool.tile([128, 512], F32, tag="ps")
                    for tap, (ky, kx) in enumerate(TAPS9):
                        base = 32 * nt2 + ky
                        rhs = x2pad[:, base:base + 31:2, kx:kx + 63:2]  # [128,16,32]
                        nc.tensor.matmul(ps[:],
                                         _mm(w3_sb[:, tap, 128 * mt:128 * mt + 128]),
                                         _mm(rhs),
                                         start=(tap == 0), stop=(tap == 8))
                    nc.scalar.activation(fs[:, mt, 512 * nt2:512 * nt2 + 512], ps[:],
                                         AF.Relu, bias=bias_sb[:, 2 + mt:3 + mt])
            nc.sync.dma_start(feat[f], fs[:])
    nc.compile()
    return nc


def build_correlator():
    nc = bacc.Bacc("TRN2", target_bir_lowering=False, debug=False, num_devices=N_CORES)
    feats = nc.dram_tensor("feats", [FPC, 128, 2, 1024], BF16, kind="ExternalInput")
    qTd = nc.dram_tensor("qTd", [128, 2, 512], BF16, kind="ExternalInput")
    cxy = nc.dram_tensor("cxy", [128, 64], F32, kind="ExternalInput")
    pred_out = nc.dram_tensor("pred_out", [4, 128, FPC, 2], F32, kind="ExternalOutput")
    occ_out = nc.dram_tensor("occ_out", [4, 128, FPC], F32, kind="ExternalOutput")

    with tile.TileContext(nc) as tc, ExitStack() as ctx:
        consts = ctx.enter_context(tc.tile_pool(name="consts", bufs=1))
        fpool = ctx.enter_context(tc.tile_pool(name="fpool", bufs=2))
        ppool = ctx.enter_context(tc.tile_pool(name="probs", bufs=2))
        spool = ctx.enter_context(tc.tile_pool(name="small", bufs=3))
        opool = ctx.enter_context(tc.tile_pool(name="outs", bufs=1))
        ps_pool = ctx.enter_context(tc.tile_pool(name="ps", bufs=3, space="PSUM"))

        qt_sb = consts.tile([128, 2, 512], BF16)
        cxy_sb = consts.tile([128, 64], F32)
        nc.sync.dma_start(qt_sb[:], qTd[:])
        nc.sync.dma_start(cxy_sb[:], cxy[:])
        predt = [opool.tile([128, FPC, 2], F32, tag=f"pred{i}", name=f"pred{i}")
                 for i in range(4)]
        occt = [opool.tile([128, FPC], F32, tag=f"occ{i}", name=f"occ{i}")
                for i in range(4)]

        for t in range(FPC):
            fsb = fpool.tile([128, 2, 1024], BF16)
            nc.sync.dma_start(fsb[:], feats[t])
            for qt in range(4):
                ps = ps_pool.tile([128, 1024], F32, tag="ps")
                for nh in range(2):
                    for kt in range(2):
                        nc.tensor.matmul(
                            ps[:, 512 * nh:512 * nh + 512],
                            _mm(qt_sb[:, kt, 128 * qt:128 * qt + 128]),
                            _mm(fsb[:, kt, 512 * nh:512 * nh + 512]),
                            start=(kt == 0), stop=(kt == 1))
                mx = spool.tile([128, 1], F32, tag="mx")
                nc.vector.reduce_max(mx[:], ps[:], axis=AX.X)
                # occlusion = sigmoid(-max(corr)/16)
                nc.scalar.activation(occt[qt][:, t:t + 1], mx[:], AF.Sigmoid,
                                     scale=-0.0625)
                nb = spool.tile([128, 1], F32, tag="nb")
                nc.scalar.mul(nb[:], mx[:], -0.625)
                probs = ppool.tile([128, 1024], BF16, tag="probs")
                den = spool.tile([128, 1], F32, tag="den")
                # exp(10/16 * (corr - max)), row sums fused
                nc.scalar.activation(probs[:], ps[:], AF.Exp, bias=nb[:],
                                     scale=0.625, accum_out=den[:])
                margs = spool.tile([128, 64], F32, tag="margs")
                nc.vector.reduce_sum(margs[:, 32:64],
                                     probs[:].rearrange("p (h w) -> p h w", h=32),
                                     axis=AX.X)
                nc.vector.reduce_sum(margs[:, 0:32],
                                     probs[:].rearrange("p (h w) -> p w h", h=32),
                                     axis=AX.X)
                wm = spool.tile([128, 64], F32, tag="wm")
                nc.vector.tensor_mul(wm[:], margs[:], cxy_sb[:])
                num2 = spool.tile([128, 2], F32, tag="num2")
                nc.vector.reduce_sum(num2[:],
                                     wm[:].rearrange("p (a c) -> p a c", a=2),
                                     axis=AX.X)
                rec = spool.tile([128, 1], F32, tag="rec")
                nc.vector.reciprocal(rec[:], den[:])
                pr2 = spool.tile([128, 2], F32, tag="pr2")
                nc.vector.tensor_mul(pr2[:], num2[:],
                                     rec[:].to_broadcast([128, 2]))
                nc.scalar.mul(predt[qt][:, t, :], pr2[:], 8.0)
        for qt in range(4):
            nc.sync.dma_start(pred_out[qt], predt[qt][:])
            nc.sync.dma_start(occ_out[qt], occt[qt][:])
    nc.compile()
    return nc


_NC_CACHE = {}


def _get_nc(name):
    if name not in _NC_CACHE:
        _NC_CACHE[name] = build_encoder() if name == "enc" else build_correlator()
    return _NC_CACHE[name]


def _run(nc, in_maps, label):
    import time
    t0 = time.monotonic()
    res = run_bass_kernel_spmd(nc, in_maps, list(range(N_CORES)))
    LAST_RUN_INFO[label] = {"wall_ns": int((time.monotonic() - t0) * 1e9)}
    return res.results


def kernel(video, query_points, w1, b1, w2, b2, w3, b3):
    video = np.ascontiguousarray(np.asarray(video, np.float32))
    qp = np.asarray(query_points, np.float32)

    # ---- host: stride-baked im2col for conv1 (p = ky*21 + kx*3 + c) ----
    vpad = np.zeros((48, 262, 262, 3), np.float32)
    vpad[:, 2:258, 2:258, :] = video.reshape(48, 256, 256, 3)
    xcol = np.zeros((48, 147, 128, 132), BF)
    for ky in range(7):
        for kx in range(7):
            sl = vpad[:, ky:ky + 256:2, kx:kx + 256:2, :]      # [48,128,128,3]
            p0 = (ky * 7 + kx) * 3
            xcol[:, p0:p0 + 3, :, :128] = sl.transpose(0, 3, 1, 2).astype(BF)

    w1f = np.asarray(w1, np.float32)[0] / np.float32(255.0)    # fold /255
    w1r = w1f.reshape(147, 64)
    w2v = np.ascontiguousarray(
        np.asarray(w2, np.float32)[0].transpose(2, 0, 1, 3).reshape(64, 9, 128))
    w3v = np.ascontiguousarray(
        np.asarray(w3, np.float32)[0].transpose(2, 0, 1, 3).reshape(128, 9, 256))
    biasv = np.zeros((128, 4), np.float32)
    biasv[:64, 0] = b1
    biasv[:, 1] = b2
    biasv[:, 2] = np.asarray(b3)[:128]
    biasv[:, 3] = np.asarray(b3)[128:]

    in_maps_A = []
    for c in range(N_CORES):
        f0 = FPC * c
        in_maps_A.append({
            "xcolA": np.ascontiguousarray(xcol[f0:f0 + FPC, :126]),
            "xcolB": np.ascontiguousarray(xcol[f0:f0 + FPC, 126:]),
            "w1A": np.ascontiguousarray(w1r[:126]).astype(BF),
            "w1B": np.ascontiguousarray(w1r[126:]).astype(BF),
            "w2s": w2v.astype(BF), "w3s": w3v.astype(BF), "biasv": biasv,
        })
    resA = _run(_get_nc("enc"), in_maps_A, "encoder")
    feat_all = np.concatenate([resA[i]["feat"] for i in range(N_CORES)], axis=0)
    # feat_all: [48, c_lo(128), c_tile(2), hw(1024)]

    # ---- host: bilinear sample query feature vectors ----
    x = feat_all.transpose(0, 3, 2, 1).reshape(48, 32, 32, 256).astype(np.float32)
    x = x.reshape(B, T, FH, FW, C)
    tq = np.clip((qp[:, :, 0] * (T - 1)).astype(np.int32), 0, T - 1)
    yq = qp[:, :, 1] * np.float32(FH - 1)
    xq = qp[:, :, 2] * np.float32(FW - 1)
    y0 = np.clip(np.floor(yq).astype(np.int32), 0, FH - 1)
    y1 = np.clip(y0 + 1, 0, FH - 1)
    x0 = np.clip(np.floor(xq).astype(np.int32), 0, FW - 1)
    x1 = np.clip(x0 + 1, 0, FW - 1)
    wy1 = (yq - y0).astype(np.float32)[..., None]
    wx1 = (xq - x0).astype(np.float32)[..., None]
    wy0 = 1.0 - wy1
    wx0 = 1.0 - wx1
    bi = np.arange(B)[:, None]
    f00 = x[bi, tq, y0, x0]
    f01 = x[bi, tq, y0, x1]
    f10 = x[bi, tq, y1, x0]
    f11 = x[bi, tq, y1, x1]
    q = (f00 * wx0 + f01 * wx1) * wy0 + (f10 * wx0 + f11 * wx1) * wy1  # [B,NQ,C]

    qTd = np.stack([
        np.ascontiguousarray(q[b].T.reshape(2, 128, 512).transpose(1, 0, 2))
        for b in range(B)
    ]).astype(BF)  # [B, 128, 2, 512]
    cxyv = np.tile(np.concatenate([np.arange(32, dtype=np.float32)] * 2)[None], (128, 1))

    in_maps_B = []
    for c in range(N_CORES):
        f0 = FPC * c
        in_maps_B.append({
            "feats": np.ascontiguousarray(feat_all[f0:f0 + FPC]),
            "qTd": qTd[f0 // T],
            "cxy": cxyv,
        })
    resB = _run(_get_nc("corr"), in_maps_B, "correlator")

    pred = np.zeros((B, NQ, T, 2), np.float32)
    occ = np.zeros((B, NQ, T), np.float32)
    for c in range(N_CORES):
        b = (FPC * c) // T
        t0 = (FPC * c) % T
        pred[b, :, t0:t0 + FPC, :] = resB[c]["pred_out"].reshape(NQ, FPC, 2)
        occ[b, :, t0:t0 + FPC] = resB[c]["occ_out"].reshape(NQ, FPC)
    return pred, occ
